# revision 17
# baseline (speedup 1.0000x reference)
"""Trainium2 Bass kernel for nn_DGEBlock (dense transformer block with
MoE-gated linears), distributed over 8 NeuronCores.

v3: fp8e4 DoubleRow matmuls for QKVO (main+gate), MLP gates, attention PV;
bf16 stationary x fp8 moving for MLP-out main (mid kept fp8-only, the mid
scale folded into Wout host-side); bf16 for MLP-in main and QK.  LN
gamma/beta folded into consuming weights so LN emits only (x-mu)*rstd.
Single persistent weight-stream pool (shared tags across phases) so weight
prefetch is never blocked by phase-pool drains.  Split AllGathers (K by
head-half, V by feature-half) hide under adjacent projection phases.
Softmax Z via fp8-DR ones-matmuls; 1/Z via reciprocal_approx_fast.  GELU
via erf (lives in the sigmoid ACT table - no table reloads in the MLP).

Sharding: data-parallel over batch (2 groups of 4 cores) x sequence-
parallel within group (512 tokens/core).
"""

import sys

for _p in ("/opt/trn_rl_repo",):
    if _p not in sys.path:
        sys.path.append(_p)

import numpy as np
import ml_dtypes

# ---------------------------------------------------------------- constants
B = 2
T = 2048
D = 2048
H = 16
HD = 128
FF = 4 * D  # 8192
EPS = 1e-5

N_CORES = 8
GROUP = 4
S = T // GROUP  # 512
P = 128
NT = D // P  # 16
NF = FF // P  # 64
NKB = T // P  # 16 key blocks per batch group
HD2 = D // 2
ISCALE = 1.0 / float(np.sqrt(HD))

WS = 64.0        # fp8 weight scale
DS = 1.0 / WS
VS = 4.0         # fp8 V scale
YS = 2.0         # fp8 y (attn out) scale
MS = 2.0         # fp8 mid (gelu out) scale; 1/MS folded into Wout host-side
EC = -2.0        # exp argument shift (cancels in softmax ratio)

RG = [[0, 1, 2, 3], [4, 5, 6, 7]]

_BF = ml_dtypes.bfloat16
_F8 = ml_dtypes.float8_e4m3

_COMPILED = None


# ------------------------------------------------------------- host prep
def _w_tiled(W, dtype, scale=1.0):
    """W [dout, din] -> [nj, 128, nt, 128] such that
    out[j, p, t, jc] == W[j*128+jc, t*128+p]  (= W^T tile (t, j))."""
    dout, din = W.shape
    nj, nt = dout // P, din // P
    Wt = W.reshape(nj, P, nt, P).transpose(0, 3, 2, 1).astype(np.float32) * scale
    if dtype is _F8:
        Wt = np.clip(Wt, -240.0, 240.0)
    return np.ascontiguousarray(Wt.astype(dtype))


def _b_cols(b, scale=1.0):
    nj = b.shape[0] // P
    return np.ascontiguousarray((b * scale).reshape(nj, P).T.astype(np.float32))


# ------------------------------------------------------------- device build
def _build():
    from concourse import bacc, tile, mybir

    fp32 = mybir.dt.float32
    bf16 = mybir.dt.bfloat16
    fp8 = mybir.dt.float8e4
    AF = mybir.ActivationFunctionType
    ALU = mybir.AluOpType
    DR = mybir.MatmulPerfMode.DoubleRow

    nc = bacc.Bacc("TRN2", target_bir_lowering=False, debug=False,
                   num_devices=N_CORES)

    # ---- I/O tensors
    xT_d = nc.dram_tensor("xT", [D, S], bf16, kind="ExternalInput")
    wd = {}
    for nm in ("Wq", "Wgq", "Wk", "Wgk", "Wo", "Wgo"):
        wd[nm] = nc.dram_tensor(nm, [NT, P, NT, P], fp8, kind="ExternalInput")
    wd["Win"] = nc.dram_tensor("Win", [NF, P, NT, P], bf16,
                               kind="ExternalInput")
    wd["Wgin"] = nc.dram_tensor("Wgin", [NF, P, NT, P], fp8,
                                kind="ExternalInput")
    wd["Wout"] = nc.dram_tensor("Wout", [NT, P, NF, P], bf16,
                                kind="ExternalInput")
    wd["Wgout"] = nc.dram_tensor("Wgout", [NT, P, NF, P], fp8,
                                 kind="ExternalInput")
    wd["WvT"] = nc.dram_tensor("WvT", [D, D], fp8, kind="ExternalInput")
    wd["WgvT"] = nc.dram_tensor("WgvT", [D, D], fp8, kind="ExternalInput")
    bvrow_d = nc.dram_tensor("bvrow", [1, D], bf16, kind="ExternalInput")
    bgvrow_d = nc.dram_tensor("bgvrow", [1, D], bf16, kind="ExternalInput")
    bd = {}
    for nm in ("bq", "bgq", "bk", "bgk", "bo", "bgo", "bout", "bgout"):
        bd[nm] = nc.dram_tensor(nm, [P, NT], fp32, kind="ExternalInput")
    for nm in ("bin", "bgin"):
        bd[nm] = nc.dram_tensor(nm, [P, NF], fp32, kind="ExternalInput")
    out_d = nc.dram_tensor("outT", [D, S], fp32, kind="ExternalOutput")

    with tile.TileContext(nc) as tc:
        with (
            tc.tile_pool(name="const", bufs=1) as constp,
            tc.tile_pool(name="bias", bufs=1) as biasp,
            tc.tile_pool(name="rows", bufs=1) as rows,
            tc.tile_pool(name="wstream", bufs=1) as ws,
            tc.tile_pool(name="epi", bufs=1) as epi,
            tc.tile_pool(name="dram", bufs=1, space="DRAM") as dramp,
        ):
            ones_col = constp.tile([P, 1], bf16)
            nc.vector.memset(ones_col[:], 1.0)
            ones_row = constp.tile([1, P], bf16)
            nc.vector.memset(ones_row[:], 1.0)
            ones2_8 = constp.tile([P, 2, 16], fp8)
            nc.vector.memset(ones2_8[:], 1.0)
            eps_t = constp.tile([1, 1], fp32)
            nc.vector.memset(eps_t[:], EPS)
            ec_t = constp.tile([P, 1], fp32)
            nc.vector.memset(ec_t[:], EC)
            bvrow = constp.tile([1, D], bf16)
            nc.sync.dma_start(bvrow[:], bvrow_d.ap())
            bgvrow = constp.tile([1, D], bf16)
            nc.sync.dma_start(bgvrow[:], bgvrow_d.ap())

            bias = {}
            for nm in bd:
                ncols = NF if nm in ("bin", "bgin") else NT
                btile = biasp.tile([P, ncols], fp32, name=f"bias_{nm}")
                nc.sync.dma_start(btile[:], bd[nm].ap())
                bias[nm] = btile

            # --- shared weight-stream tiles (persistent pool, shared tags)
            def w8_16(nm):
                return ws.tile([P, NT, P], fp8, tag="w8_16", name=nm, bufs=3)

            def wbf_16(nm):
                return ws.tile([P, NT, P], bf16, tag="wbf_16", name=nm,
                               bufs=3)

            def w8_32(nm):
                return ws.tile([P, 32, P], fp8, tag="w8_32", name=nm, bufs=2)

            def wbf_32(nm):
                return ws.tile([P, 32, P], bf16, tag="wbf_32", name=nm,
                               bufs=2)

            def wv8(nm):
                return ws.tile([P, NT // 2, 4 * P], fp8, tag="wv8", name=nm,
                               bufs=2)

            # --- shared epilogue temps
            def sig_t(nm):
                return epi.tile([P, S], bf16, tag="sig", name=nm, bufs=2)

            def tmpm_t(nm):
                return epi.tile([P, S], bf16, tag="tmpm", name=nm, bufs=2)

            def u_t(nm):
                return epi.tile([P, S], bf16, tag="u", name=nm, bufs=2)

            def e_t(nm):
                return epi.tile([P, S], bf16, tag="e", name=nm, bufs=2)

            def kv_t(nm):
                return epi.tile([P, S], bf16, tag="kv", name=nm, bufs=2)

            def outf_t(nm):
                return epi.tile([P, S], fp32, tag="outf", name=nm, bufs=2)

            def vout_t(nm):
                return epi.tile([P, S], fp8, tag="vout", name=nm, bufs=2)

            # ---------- LN helper (gamma/beta pre-folded into weights) ----
            def ln_T(src, hpool, tmpool, psln, name, out_dtypes):
                """src: SBUF [128, NT, S] bf16.  Returns z=(x-mu)*rstd in
                the dtypes listed in out_dtypes (one tile per dtype)."""
                sq = tmpool.tile([P, NT, S], bf16, name=f"{name}_sq")
                S1 = psln.tile([1, S], fp32, name=f"{name}_S1", tag="ln_S1")
                S2 = psln.tile([1, S], fp32, name=f"{name}_S2", tag="ln_S2")
                for t in range(NT):
                    nc.scalar.activation(sq[:, t, :], src[:, t, :], AF.Square)
                for t in range(NT):
                    nc.tensor.matmul(S1[:], ones_col[:], src[:, t, :],
                                     start=(t == 0), stop=(t == NT - 1))
                for t in range(NT):
                    nc.tensor.matmul(S2[:], ones_col[:], sq[:, t, :],
                                     start=(t == 0), stop=(t == NT - 1))

                def row(nm, dt=fp32):
                    return rows.tile([1, S], dt, name=f"{name}_{nm}",
                                     tag=f"ln_{nm}")

                mean = row("mean")
                nc.vector.tensor_scalar_mul(mean[:], S1[:], 1.0 / D)
                m2 = row("m2")
                nc.vector.tensor_scalar_mul(m2[:], S2[:], 1.0 / D)
                msq = row("msq")
                nc.vector.tensor_tensor(msq[:], mean[:], mean[:],
                                        op=ALU.mult)
                var = row("var")
                nc.vector.tensor_tensor(var[:], m2[:], msq[:],
                                        op=ALU.subtract)
                std = row("std")
                nc.scalar.activation(std[:], var[:], AF.Sqrt, bias=eps_t[:])
                rstd = row("rstd")
                nc.vector.reciprocal_approx_fast(rstd[:], std[:])
                rstd_bf = row("rstdbf", bf16)
                nc.gpsimd.tensor_copy(rstd_bf[:], rstd[:])
                mr_bf = row("mrbf", bf16)
                nc.vector.tensor_tensor(mr_bf[:], mean[:], rstd[:],
                                        op=ALU.mult)
                Ab_p = psln.tile([P, S], fp32, name=f"{name}_Abp",
                                 tag="ln_Abp")
                nc.tensor.matmul(Ab_p[:], ones_row[:], rstd_bf[:])
                Bb_p = psln.tile([P, S], fp32, name=f"{name}_Bbp",
                                 tag="ln_Bbp")
                nc.tensor.matmul(Bb_p[:], ones_row[:], mr_bf[:])
                Ab = tmpool.tile([P, S], bf16, name=f"{name}_Ab")
                nc.vector.tensor_copy(Ab[:], Ab_p[:])
                Bb = tmpool.tile([P, S], bf16, name=f"{name}_Bb")
                nc.vector.tensor_copy(Bb[:], Bb_p[:])
                outs = [hpool.tile([P, NT, S], dt, name=f"{name}_h{i}")
                        for i, dt in enumerate(out_dtypes)]
                for t in range(NT):
                    tmp = tmpool.tile([P, S], bf16, name=f"{name}_t0_{t}",
                                      tag="ln_t0", bufs=3)
                    nc.vector.tensor_tensor(tmp[:], src[:, t, :], Ab[:],
                                            op=ALU.mult)
                    nc.vector.scalar_tensor_tensor(outs[0][:, t, :],
                                                   tmp[:], 0.0, Bb[:],
                                                   op0=ALU.add,
                                                   op1=ALU.subtract)
                    for o in outs[1:]:
                        nc.gpsimd.tensor_copy(o[:, t, :], outs[0][:, t, :])
                return outs

            # ---------- fp8 DoubleRow gated projection (T-layout) --------
            def proj_gated8(src8, nt, nj, wname, wgname, pspool, epilogue):
                for j in range(nj):
                    main = pspool.tile([P, S], fp32, name=f"{wname}_m{j}",
                                       tag="pj_main", bufs=2)
                    gate = pspool.tile([P, S], fp32, name=f"{wname}_g{j}",
                                       tag="pj_gate", bufs=2)
                    wt = w8_16(f"w_{wname}_{j}")
                    nc.sync.dma_start(wt[:], wd[wname].ap()[j])
                    for t in range(0, nt, 2):
                        nc.tensor.matmul(main[:], wt[:, t:t + 2, :],
                                         src8[:, t:t + 2, :],
                                         start=(t == 0), stop=(t == nt - 2),
                                         perf_mode=DR)
                    wg = w8_16(f"w_{wgname}_{j}")
                    nc.sync.dma_start(wg[:], wd[wgname].ap()[j])
                    for t in range(0, nt, 2):
                        nc.tensor.matmul(gate[:], wg[:, t:t + 2, :],
                                         src8[:, t:t + 2, :],
                                         start=(t == 0), stop=(t == nt - 2),
                                         perf_mode=DR)
                    epilogue(j, main, gate)

            with tc.tile_pool(name="x2p", bufs=1) as x2p:
              with tc.tile_pool(name="xt", bufs=1) as xtp:
                xbf = xtp.tile([P, NT, S], bf16)
                nc.sync.dma_start(
                    xbf[:], xT_d.ap().rearrange("(t p) s -> p t s", p=P))

                kb_half = [dramp.tile([HD2, S], bf16, name=f"kb{i}")
                           for i in range(2)]
                kg_half = [dramp.tile([GROUP * HD2, S], bf16, name=f"kg{i}")
                           for i in range(2)]
                vb_half = [dramp.tile([S, HD2], fp8, name=f"vb{i}")
                           for i in range(2)]
                vg_half = [dramp.tile([GROUP * S, HD2], fp8, name=f"vg{i}")
                           for i in range(2)]

                with tc.tile_pool(name="yp", bufs=1) as ypool:
                  y8 = ypool.tile([P, H, S], fp8, name="y8")
                  with tc.tile_pool(name="qp", bufs=1) as qpool:
                    q = qpool.tile([P, NT, S], bf16)

                    with tc.tile_pool(name="hq", bufs=1) as hqp:
                        with (
                            tc.tile_pool(name="ln1tmp", bufs=1) as ln1tmp,
                            tc.tile_pool(name="ln1ps", bufs=1,
                                         space="PSUM") as ln1ps,
                        ):
                            (h1f8,) = ln_T(xbf, hqp, ln1tmp, ln1ps, "ln1",
                                           [fp8])

                        # ---- K projection + split AllGather ----
                        with tc.tile_pool(name="pjps", bufs=1,
                                          space="PSUM") as pjps:
                            def k_epi(j, main, gate):
                                sig = sig_t(f"sig_k_{j}")
                                nc.scalar.activation(
                                    sig[:], gate[:], AF.Sigmoid,
                                    bias=bias["bgk"][:, j:j + 1], scale=DS)
                                tmpm = tmpm_t(f"tmpm_k_{j}")
                                nc.scalar.activation(
                                    tmpm[:], main[:], AF.Identity,
                                    bias=bias["bk"][:, j:j + 1], scale=DS)
                                kv = kv_t(f"kv_k_{j}")
                                nc.vector.tensor_tensor(kv[:], tmpm[:],
                                                        sig[:], op=ALU.mult)
                                half, jj = divmod(j, NT // 2)
                                nc.scalar.dma_start(
                                    kb_half[half][jj * P:(jj + 1) * P, :],
                                    kv[:])
                                if j == NT // 2 - 1 or j == NT - 1:
                                    nc.gpsimd.collective_compute(
                                        "AllGather", ALU.bypass,
                                        ins=[kb_half[half][:]],
                                        outs=[kg_half[half][:]],
                                        replica_groups=RG)

                            proj_gated8(h1f8, NT, NT, "Wk", "Wgk",
                                        pjps, k_epi)

                        # ---- V projection, N-layout, fp8 DR ----
                        with tc.tile_pool(name="vps", bufs=1,
                                          space="PSUM") as vps:
                            TC = NT // 2  # 8 k-subtiles per weight chunk
                            wv_v = wd["WvT"].ap().rearrange(
                                "(c t p) s -> c p t s", t=TC, p=P)
                            wgv_v = wd["WgvT"].ap().rearrange(
                                "(c t p) s -> c p t s", t=TC, p=P)
                            for n in range(4):
                                vmain = [vps.tile([P, S], fp32,
                                                  tag="v_main", bufs=4,
                                                  name=f"vm_{n}_{m}")
                                         for m in range(4)]
                                vgate = [vps.tile([P, S], fp32,
                                                  tag="v_gate", bufs=4,
                                                  name=f"vg_{n}_{m}")
                                         for m in range(4)]
                                for ci in range(2):
                                    wvt = wv8(f"wv_{n}_{ci}")
                                    nc.sync.dma_start(
                                        wvt[:],
                                        wv_v[ci, :, :, n * S:(n + 1) * S])
                                    wgvt = wv8(f"wgv_{n}_{ci}")
                                    nc.sync.dma_start(
                                        wgvt[:],
                                        wgv_v[ci, :, :, n * S:(n + 1) * S])
                                    for m in range(4):
                                        for ti in range(0, TC, 2):
                                            t = ci * TC + ti
                                            nc.tensor.matmul(
                                                vmain[m][:],
                                                h1f8[:, t:t + 2,
                                                     m * P:(m + 1) * P],
                                                wvt[:, ti:ti + 2, :],
                                                start=(t == 0), stop=False,
                                                perf_mode=DR)
                                        for ti in range(0, TC, 2):
                                            t = ci * TC + ti
                                            nc.tensor.matmul(
                                                vgate[m][:],
                                                h1f8[:, t:t + 2,
                                                     m * P:(m + 1) * P],
                                                wgvt[:, ti:ti + 2, :],
                                                start=(t == 0), stop=False,
                                                perf_mode=DR)
                                for m in range(4):
                                    nc.tensor.matmul(
                                        vmain[m][:], ones_row[:],
                                        bvrow[:, n * S:(n + 1) * S],
                                        start=False, stop=True)
                                    nc.tensor.matmul(
                                        vgate[m][:], ones_row[:],
                                        bgvrow[:, n * S:(n + 1) * S],
                                        start=False, stop=True)
                                    vsig = sig_t(f"vsig_{n}_{m}")
                                    nc.scalar.activation(vsig[:],
                                                         vgate[m][:],
                                                         AF.Sigmoid,
                                                         scale=DS)
                                    vout = vout_t(f"vout_{n}_{m}")
                                    nc.vector.scalar_tensor_tensor(
                                        vout[:], vmain[m][:], VS * DS,
                                        vsig[:], op0=ALU.mult,
                                        op1=ALU.mult)
                                    half = n // 2
                                    nc.scalar.dma_start(
                                        vb_half[half][
                                            m * P:(m + 1) * P,
                                            (n % 2) * S:(n % 2 + 1) * S],
                                        vout[:])
                                if n == 1 or n == 3:
                                    half = n // 2
                                    nc.gpsimd.collective_compute(
                                        "AllGather", ALU.bypass,
                                        ins=[vb_half[half][:]],
                                        outs=[vg_half[half][:]],
                                        replica_groups=RG)

                        # ---- Q projection (standalone) ----
                        with tc.tile_pool(name="qps", bufs=1,
                                          space="PSUM") as qps:
                            def q_epi(j, main, gate):
                                sig = sig_t(f"sig_q_{j}")
                                nc.scalar.activation(
                                    sig[:], gate[:], AF.Sigmoid,
                                    bias=bias["bgq"][:, j:j + 1], scale=DS)
                                tmpm = tmpm_t(f"tmpm_q_{j}")
                                nc.scalar.activation(
                                    tmpm[:], main[:], AF.Identity,
                                    bias=bias["bq"][:, j:j + 1], scale=DS)
                                nc.vector.tensor_tensor(q[:, j, :],
                                                        tmpm[:], sig[:],
                                                        op=ALU.mult)

                            proj_gated8(h1f8, NT, NT, "Wq", "Wgq",
                                        qps, q_epi)

                    # ---- attention ----
                    with (
                        tc.tile_pool(name="vres", bufs=1) as vresp,
                        tc.tile_pool(name="kstream", bufs=2) as kpool,
                        tc.tile_pool(name="apool", bufs=2) as apool,
                        tc.tile_pool(name="atps", bufs=1,
                                     space="PSUM") as atps,
                    ):
                        Vt = vresp.tile([P, NKB, D], fp8)

                        def load_vt_half(half):
                            nc.gpsimd.dma_start(
                                Vt[:, :, half * HD2:(half + 1) * HD2],
                                vg_half[half][:].rearrange(
                                    "(kb p) c -> p kb c", p=P))

                        load_vt_half(0)

                        def attn_head(hh):
                            if hh == 2:
                                load_vt_half(1)
                            half, hl = divmod(hh, H // 2)
                            Kh = kpool.tile([P, GROUP, S], bf16, tag="Kh",
                                            name=f"Kh_{hh}")
                            kgv = kg_half[half][:].rearrange(
                                "(g r p) s -> r p g s", g=GROUP, p=P)
                            nc.gpsimd.dma_start(Kh[:], kgv[hl])
                            At = apool.tile([P, NKB, S], fp8, tag="At",
                                            name=f"At_{hh}")
                            Yp = atps.tile([P, S], fp32, name=f"Y_{hh}",
                                           tag="Yp", bufs=2)
                            Zp = atps.tile([16, S], fp32, name=f"Z_{hh}",
                                           tag="Zp", bufs=2)
                            for kp in range(8):
                                kb = 2 * kp
                                Lp2 = atps.tile([P, 2, S], fp32,
                                                name=f"L_{hh}_{kp}",
                                                tag="logits", bufs=2)
                                nc.tensor.matmul(
                                    Lp2[:, 0, :],
                                    Kh[:, kb // 4,
                                       (kb % 4) * P:(kb % 4 + 1) * P],
                                    q[:, hh, :])
                                nc.tensor.matmul(
                                    Lp2[:, 1, :],
                                    Kh[:, (kb + 1) // 4,
                                       ((kb + 1) % 4) * P:
                                       ((kb + 1) % 4 + 1) * P],
                                    q[:, hh, :])
                                nc.scalar.activation(
                                    At[:, kb:kb + 2, :], Lp2[:, :, :],
                                    AF.Exp, bias=ec_t[:], scale=ISCALE)
                                nc.tensor.matmul(
                                    Yp[:], Vt[:, kb:kb + 2,
                                              hh * P:(hh + 1) * P],
                                    At[:, kb:kb + 2, :],
                                    start=(kp == 0), stop=(kp == 7),
                                    perf_mode=DR)
                                nc.tensor.matmul(
                                    Zp[:], ones2_8[:, :, :],
                                    At[:, kb:kb + 2, :],
                                    start=(kp == 0), stop=(kp == 7),
                                    perf_mode=DR)
                            urow = rows.tile([1, S], fp32, name=f"u_{hh}",
                                             tag="urow", bufs=2)
                            nc.vector.reciprocal_approx_fast(urow[:],
                                                             Zp[0:1, :])
                            ubf = rows.tile([1, S], bf16, name=f"ubf_{hh}",
                                            tag="ubf", bufs=2)
                            nc.gpsimd.tensor_copy(ubf[:], urow[:])
                            Up = atps.tile([P, 2, S], fp32, name=f"Up_{hh}",
                                           tag="logits", bufs=2)
                            nc.tensor.matmul(Up[:, 0, :], ones_row[:],
                                             ubf[:])
                            Us = u_t(f"Us_{hh}")
                            nc.vector.tensor_copy(Us[:], Up[:, 0, :])
                            # y8 = fp8(YS * Yp * u / VS)
                            nc.vector.scalar_tensor_tensor(
                                y8[:, hh, :], Yp[:], YS / VS, Us[:],
                                op0=ALU.mult, op1=ALU.mult)

                        for hh in range(H):
                            attn_head(hh)

                  # ---- o-proj + residual ----
                  x2 = x2p.tile([P, NT, S], bf16, name="x2")
                  with tc.tile_pool(name="pj2ps", bufs=1,
                                    space="PSUM") as pj2ps:
                      def o_epi(j, main, gate):
                          sig = sig_t(f"sig_o_{j}")
                          nc.scalar.activation(
                              sig[:], gate[:], AF.Sigmoid,
                              bias=bias["bgo"][:, j:j + 1], scale=DS / YS)
                          tmpm = tmpm_t(f"tmpm_o_{j}")
                          nc.scalar.activation(
                              tmpm[:], main[:], AF.Identity,
                              bias=bias["bo"][:, j:j + 1], scale=DS / YS)
                          yo = kv_t(f"o_yo_{j}")
                          nc.vector.tensor_tensor(yo[:], tmpm[:], sig[:],
                                                  op=ALU.mult)
                          nc.vector.tensor_tensor(x2[:, j, :], yo[:],
                                                  xbf[:, j, :], op=ALU.add)

                      proj_gated8(y8, H, NT, "Wo", "Wgo", pj2ps, o_epi)

              # ---- LN2 + MLP ----
              with tc.tile_pool(name="midp", bufs=1) as midp:
                  mid8 = midp.tile([P, NF, S], fp8, name="mid8")
                  with tc.tile_pool(name="h2p", bufs=1) as h2p:
                      with (
                          tc.tile_pool(name="ln2tmp", bufs=1) as ln2tmp,
                          tc.tile_pool(name="ln2ps", bufs=1,
                                       space="PSUM") as ln2ps,
                      ):
                          h2bf, h2f8 = ln_T(x2, h2p, ln2tmp, ln2ps, "ln2",
                                            [bf16, fp8])

                      with tc.tile_pool(name="m1ps", bufs=1,
                                        space="PSUM") as m1ps:
                          for j in range(NF):
                              main = m1ps.tile([P, S], fp32,
                                               name=f"in_m{j}",
                                               tag="pj_main", bufs=2)
                              gate = m1ps.tile([P, S], fp32,
                                               name=f"in_g{j}",
                                               tag="pj_gate", bufs=2)
                              wt = wbf_16(f"w_Win_{j}")
                              nc.sync.dma_start(wt[:], wd["Win"].ap()[j])
                              for t in range(NT):
                                  nc.tensor.matmul(main[:], wt[:, t, :],
                                                   h2bf[:, t, :],
                                                   start=(t == 0),
                                                   stop=(t == NT - 1))
                              wg = w8_16(f"w_Wgin_{j}")
                              nc.sync.dma_start(wg[:], wd["Wgin"].ap()[j])
                              for t in range(0, NT, 2):
                                  nc.tensor.matmul(gate[:],
                                                   wg[:, t:t + 2, :],
                                                   h2f8[:, t:t + 2, :],
                                                   start=(t == 0),
                                                   stop=(t == NT - 2),
                                                   perf_mode=DR)
                              sig = sig_t(f"sig_in_{j}")
                              nc.scalar.activation(
                                  sig[:], gate[:], AF.Sigmoid,
                                  bias=bias["bgin"][:, j:j + 1], scale=DS)
                              tmpm = tmpm_t(f"tmpm_in_{j}")
                              nc.vector.scalar_tensor_tensor(
                                  tmpm[:], main[:],
                                  bias["bin"][:, j:j + 1], sig[:],
                                  op0=ALU.add, op1=ALU.mult)
                              # exact gelu: 0.5*x*(1+erf(x/sqrt2)); with
                              # mid8 = MS*mid and MS=2: mid8 = tmpm*(1+erf)
                              e = e_t(f"erf_{j}")
                              nc.scalar.activation(
                                  e[:], tmpm[:], AF.Erf,
                                  scale=0.7071067811865476)
                              u = u_t(f"gelu_u_{j}")
                              nc.vector.tensor_scalar_add(u[:], e[:], 1.0)
                              nc.vector.tensor_tensor(mid8[:, j, :],
                                                      tmpm[:], u[:],
                                                      op=ALU.mult)

                  with tc.tile_pool(name="m2ps", bufs=1,
                                    space="PSUM") as m2ps:
                      TCO = 32
                      for j in range(NT):
                          main = m2ps.tile([P, S], fp32, name=f"out_m{j}",
                                           tag="pj_main", bufs=2)
                          gate = m2ps.tile([P, S], fp32, name=f"out_g{j}",
                                           tag="pj_gate", bufs=2)
                          for ci in range(2):
                              wt = wbf_32(f"w_Wout_{j}_{ci}")
                              nc.sync.dma_start(
                                  wt[:],
                                  wd["Wout"].ap()[j, :,
                                                  ci * TCO:(ci + 1) * TCO,
                                                  :])
                              for ti in range(TCO):
                                  t = ci * TCO + ti
                                  # bf16 weights x fp8 moving (mid8); the
                                  # 1/MS descale is folded into Wout
                                  nc.tensor.matmul(main[:], wt[:, ti, :],
                                                   mid8[:, t, :],
                                                   start=(t == 0),
                                                   stop=(t == NF - 1))
                          for ci in range(2):
                              wg = w8_32(f"w_Wgout_{j}_{ci}")
                              nc.sync.dma_start(
                                  wg[:],
                                  wd["Wgout"].ap()[j, :,
                                                   ci * TCO:(ci + 1) * TCO,
                                                   :])
                              for ti in range(0, TCO, 2):
                                  t = ci * TCO + ti
                                  nc.tensor.matmul(gate[:],
                                                   wg[:, ti:ti + 2, :],
                                                   mid8[:, t:t + 2, :],
                                                   start=(t == 0),
                                                   stop=(t == NF - 2),
                                                   perf_mode=DR)
                          sig = sig_t(f"sig_out_{j}")
                          nc.scalar.activation(
                              sig[:], gate[:], AF.Sigmoid,
                              bias=bias["bgout"][:, j:j + 1],
                              scale=DS / MS)
                          tmpm = tmpm_t(f"tmpm_out_{j}")
                          nc.vector.scalar_tensor_tensor(
                              tmpm[:], main[:], bias["bout"][:, j:j + 1],
                              sig[:], op0=ALU.add, op1=ALU.mult)
                          outf = outf_t(f"out_f_{j}")
                          nc.vector.tensor_tensor(outf[:], tmpm[:],
                                                  x2[:, j, :], op=ALU.add)
                          nc.scalar.dma_start(
                              out_d.ap()[j * P:(j + 1) * P, :], outf[:])

    nc.compile()
    return nc


def _prep_shared_inputs(inputs):
    g1 = np.asarray(inputs["ln1_g"], np.float32)
    b1 = np.asarray(inputs["ln1_b"], np.float32)
    g2 = np.asarray(inputs["ln2_g"], np.float32)
    b2 = np.asarray(inputs["ln2_b"], np.float32)

    def W(nm):
        return np.asarray(inputs[nm], np.float32)

    m = {}
    for nm, src in (("Wq", "W_q"), ("Wgq", "Wg_q"), ("Wk", "W_k"),
                    ("Wgk", "Wg_k")):
        m[nm] = _w_tiled(W(src) * g1[None, :], _F8, WS)
    for nm, src in (("Wo", "W_o"), ("Wgo", "Wg_o")):
        m[nm] = _w_tiled(W(src), _F8, WS)
    m["Win"] = _w_tiled(W("W_in") * g2[None, :], _BF)
    m["Wgin"] = _w_tiled(W("Wg_in") * g2[None, :], _F8, WS)
    # mid is shipped as fp8(MS*mid); compensate exactly in bf16 weights
    m["Wout"] = _w_tiled(W("W_out"), _BF, 1.0 / MS)
    m["Wgout"] = _w_tiled(W("Wg_out"), _F8, WS)
    wv = W("W_v") * g1[None, :]
    wgv = W("Wg_v") * g1[None, :]
    m["WvT"] = np.ascontiguousarray(
        np.clip(wv.T * WS, -240, 240).astype(_F8))
    m["WgvT"] = np.ascontiguousarray(
        np.clip(wgv.T * WS, -240, 240).astype(_F8))

    def bias_fold(bname, Wname, beta):
        return np.asarray(inputs[bname], np.float32) + W(Wname) @ beta

    m["bvrow"] = (bias_fold("b_v", "W_v", b1) * WS).astype(_BF).reshape(1, D)
    m["bgvrow"] = (bias_fold("bg_v", "Wg_v", b1) * WS).astype(_BF).reshape(
        1, D)
    m["bq"] = _b_cols(bias_fold("b_q", "W_q", b1))
    m["bgq"] = _b_cols(bias_fold("bg_q", "Wg_q", b1))
    m["bk"] = _b_cols(bias_fold("b_k", "W_k", b1))
    m["bgk"] = _b_cols(bias_fold("bg_k", "Wg_k", b1))
    m["bo"] = _b_cols(np.asarray(inputs["b_o"], np.float32))
    m["bgo"] = _b_cols(np.asarray(inputs["bg_o"], np.float32))
    m["bin"] = _b_cols(bias_fold("b_in", "W_in", b2))
    m["bgin"] = _b_cols(bias_fold("bg_in", "Wg_in", b2))
    m["bout"] = _b_cols(np.asarray(inputs["b_out"], np.float32))
    m["bgout"] = _b_cols(np.asarray(inputs["bg_out"], np.float32))
    return m


def _install_trace_shim():
    """Provide antenv.axon_hooks (NTFF profiling) if the image lacks it."""
    import contextlib
    import ctypes
    import types

    try:
        import antenv.axon_hooks  # noqa: F401
        return
    except ImportError:
        pass
    try:
        import antenv
    except ImportError:
        return
    so_path = "/opt/axon/libaxon_pjrt.so"
    try:
        lib = ctypes.CDLL(so_path)
    except OSError:
        return
    if not hasattr(lib, "axon_start_nrt_profile"):
        return
    lib.axon_start_nrt_profile.argtypes = [ctypes.POINTER(ctypes.c_int64),
                                           ctypes.c_size_t]
    lib.axon_start_nrt_profile.restype = ctypes.c_int64
    lib.axon_stop_nrt_profile.argtypes = [ctypes.c_char_p]
    lib.axon_stop_nrt_profile.restype = ctypes.c_int64

    @contextlib.contextmanager
    def hook(output_dir, device_ids):
        import jax

        jax.devices()
        if device_ids:
            ids = (ctypes.c_int64 * len(device_ids))(*device_ids)
            rc = lib.axon_start_nrt_profile(ids, len(device_ids))
        else:
            rc = lib.axon_start_nrt_profile(None, 0)
        if rc != 0:
            raise RuntimeError(f"axon_start_nrt_profile rc={rc}")
        try:
            yield
        finally:
            n = lib.axon_stop_nrt_profile(str(output_dir).encode())
            print(f"profile: {n} ntff file(s) in {output_dir}",
                  file=sys.stderr)

    mod = types.ModuleType("antenv.axon_hooks")
    mod.get_axon_ntff_profile_hook = lambda: hook
    mod.set_axon_ntff_profile_hook = lambda h: None
    sys.modules["antenv.axon_hooks"] = mod
    antenv.axon_hooks = mod


LAST_RESULTS = None


def kernel(_trace=False, **inputs):
    global _COMPILED, LAST_RESULTS
    from concourse import bass_utils

    if _trace:
        _install_trace_shim()

    if _COMPILED is None:
        _COMPILED = _build()
    nc = _COMPILED

    shared = _prep_shared_inputs(inputs)
    x = np.asarray(inputs["x"], dtype=np.float32)  # [B, T, D]
    in_maps = []
    for c in range(N_CORES):
        g, s = divmod(c, GROUP)
        xT_c = np.ascontiguousarray(x[g, s * S:(s + 1) * S, :].T.astype(_BF))
        m = dict(shared)
        m["xT"] = xT_c
        in_maps.append(m)

    LAST_RESULTS = bass_utils.run_bass_kernel_spmd(
        nc, in_maps, core_ids=list(range(N_CORES)), trace=_trace)

    out = np.empty((B, T, D), dtype=np.float32)
    for c in range(N_CORES):
        g, s = divmod(c, GROUP)
        out[g, s * S:(s + 1) * S, :] = LAST_RESULTS.results[c]["outT"].T
    return out
